# revision 1
# baseline (speedup 1.0000x reference)
"""KalmanNetNN Trainium2 kernel: 8-core tensor-parallel, SBUF-resident bf16 weights.

Design:
- T=512 strictly sequential steps; per step a chain of GEMVs (W1 4160x52,
  W_ih 6960x4160, W_hh 6960x2320, W2 768x2320, W3 192x768) + tiny Kalman update.
- Weights sharded across 8 cores, resident in SBUF as pre-transposed bf16
  stationary tiles (W-stationary GEMV: out[128,1] tiles land in clean layout).
- GRU hidden (2320) sharded 290/core, padded to 384 slots (3 cols of 128).
- Per step one AllGather exchanges [h_own(384) | l2_partial(768)] bf16;
  W2 is column-sharded so l2 partials sum locally after the AG.
- Small Kalman recurrence (A, C, norms, kg apply) in fp32, replicated on all
  cores (the A-recurrence is unstable; fp32 there keeps rel err ~1e-7).
"""

import numpy as np
import ml_dtypes

M, N, T = 4, 48, 512
D_IN = M + N            # 52
H1 = 4160               # l1 dim
HID = 2320              # GRU hidden
H2 = 768                # l2 dim
DOUT = M * N            # 192

NCORES = 8
SLOTS = 384             # per-core padded h slots (3 cols of 128)
OWN = HID // NCORES     # 290 real h per core
CH = 3 * NCORES         # 24 global h cols
H1P = 4224              # l1 padded (33 cols); slot (127,32) = bias-1
MO1 = H1P // 128        # 33
MOG = 9                 # gi/gh out cols (3 gates x 3 cols)
MO2 = H2 // 128         # 6
DOP = 256               # padded kg rows
MO3 = DOP // 128        # 2

BF = ml_dtypes.bfloat16
CHUNK = 16
NSTEPS = T


def _tile_stationary(Wc, Mo, C):
    """Wc [Mo*128, C*128] -> [128, Mo*C*128] with tile (m,k) at (m*C+k)*128.
    lhsT[p, j] of tile (m,k) = Wc[128m+j, 128k+p]."""
    A = Wc.reshape(Mo, 128, C, 128)          # m, j, k, p
    A = np.transpose(A, (3, 0, 2, 1))        # p, m, k, j
    return np.ascontiguousarray(A.reshape(128, Mo * C * 128))


def _prep_core(c, A, C_, x0, h0, y_seq, W1, b1, W_ih, W_hh, b_ih, b_hh, W2, b2, W3, b3):
    f32 = np.float32
    out = {}

    # --- W1 | b1: knet layout [97]: dy 0-47, dx 64-67, bias-1 at 96
    W1b = np.zeros((H1P, 97), f32)
    W1b[:H1, 0:N] = W1[:, 0:N]
    W1b[:H1, 64:64 + M] = W1[:, N:D_IN]
    W1b[:H1, 96] = b1
    W1b[H1P - 1, 96] = 1.0   # l1[4223] = relu(1*knet[96]) = 1 -> bias-1 slot
    A1 = W1b.reshape(MO1, 128, 1, 97)
    A1 = np.transpose(A1, (3, 0, 2, 1)).reshape(97, MO1 * 128)
    out["w1t"] = np.ascontiguousarray(A1).astype(BF)

    # --- per-core gate rows: rho = g*384 + s (s<290 real)
    rows = np.zeros((3 * SLOTS,), np.int64) - 1
    for g in range(3):
        for s in range(OWN):
            rows[g * SLOTS + s] = g * HID + c * OWN + s
    valid = rows >= 0

    # --- W_ih core [1152, H1P] + b_ih in col 4223 (l1 bias-1 slot)
    Wih = np.zeros((3 * SLOTS, H1P), f32)
    Wih[valid, :H1] = W_ih[rows[valid]]
    Wih[valid, H1P - 1] = b_ih[rows[valid]]
    Wih[SLOTS + 383, H1P - 1] = 30.0  # z-gate bias at dead slot s=383 -> z=1, h slot stays 1
    out["wih"] = _tile_stationary(Wih, MOG, MO1).astype(BF)

    # --- global h slot map: slot (cc, j, p) -> h index cc*290 + 128j + p (if <290)
    hidx = np.zeros((CH, 128), np.int64) - 1    # [col, p]
    for cc in range(NCORES):
        for j in range(3):
            for p in range(128):
                s = 128 * j + p
                if s < OWN:
                    hidx[3 * cc + j, p] = cc * OWN + s
    # --- W_hh core [1152, CH*128] + b_hh at slot col 23, p 127 (h bias-1)
    Whh = np.zeros((3 * SLOTS, CH * 128), f32)
    flat = hidx.reshape(-1)                      # [CH*128] in (col, p) order
    vv = flat >= 0
    Whh[np.ix_(valid, vv)] = W_hh[np.ix_(rows[valid], flat[vv])]
    Whh[valid, 23 * 128 + 127] = b_hh[rows[valid]]
    # reorder cols from (col,p) to matmul layout: contraction chunk k uses partition p
    # stationary tile (m,k): lhsT[p, j] = Whh[128m+j, slot(k, p)] ; slot(k,p) flat idx k*128+p
    out["whh"] = _tile_stationary(Whh, MOG, CH).astype(BF)

    # --- W2 column shard: own slots only [768, 3*128]
    W2c = np.zeros((H2, 3 * 128), f32)
    for j in range(3):
        for p in range(128):
            s = 128 * j + p
            if s < OWN:
                W2c[:, j * 128 + p] = W2[:, c * OWN + s]
    out["w2c"] = _tile_stationary(W2c, MO2, 3).astype(BF)

    # --- W3: rows rho=4n+m <-> W3 row m*N+n, x 1e-4 fold
    W3s = np.zeros((DOP, H2), f32)
    for rho in range(DOUT):
        n_, m_ = rho // 4, rho % 4
        W3s[rho] = W3[m_ * N + n_] * 1e-4
    out["w3s"] = _tile_stationary(W3s, MO3, MO2).astype(BF)

    # --- small fp32 constants
    CA = (C_[:, :M] @ A).astype(f32)
    c5 = C_[:, M].astype(f32)
    S1 = np.zeros((M + 1, 112), f32)   # pk: x_prior @ 0-3, m1y @ 64-111
    S1[:M, :M] = A.T
    S1[:M, 64:] = CA.T
    S1[M, 64:] = c5
    out["s1"] = S1
    S2 = np.zeros((96, 2), f32)
    S2[:N, 0] = 1.0
    S2[64:64 + M, 1] = 1.0
    out["s2"] = S2
    BB = np.zeros((2, 96), f32)
    BB[0, :N] = 1.0
    BB[1, 64:64 + M] = 1.0
    out["bb"] = BB
    E = np.zeros((DOP, 48), f32)
    for rho in range(DOUT):
        E[rho, rho // 4] = 1.0
    out["e01"] = np.ascontiguousarray(E.reshape(2, 128, 48).transpose(2, 0, 1).reshape(48, 256))
    S4 = np.zeros((128, M), f32)
    for p in range(128):
        S4[p, p % 4] = 1.0
    out["s4"] = S4
    b2s = np.zeros((128, MO2), f32)
    b2s[:, :] = b2.reshape(MO2, 128).T
    out["b2s"] = b2s
    b3v = np.zeros((DOP,), f32)
    for rho in range(DOUT):
        n_, m_ = rho // 4, rho % 4
        b3v[rho] = b3[m_ * N + n_] * 1e-4
    out["b3s"] = np.ascontiguousarray(b3v.reshape(MO3, 128).T)
    out["epsv"] = np.full((2, 1), 1e-24, f32)

    # --- h0 blocks (all cores' slots) bf16 + own fp32
    h0b = np.zeros((128, CH), f32)
    for cc in range(CH):
        for p in range(128):
            if hidx[cc, p] >= 0:
                h0b[p, cc] = h0[hidx[cc, p]]
    h0b[127, 23] = 1.0  # bias-1 slot
    out["h0b"] = h0b.astype(BF)
    own = np.ascontiguousarray(h0b[:, 3 * c:3 * c + 3]).astype(f32)
    own[127, 2] = 1.0
    out["h0own"] = own
    return out


def _build(nc):
    import concourse.bass as bass
    import concourse.mybir as mybir
    import concourse.tile as tile

    dt = mybir.dt
    AF = mybir.ActivationFunctionType
    ds = bass.ds

    # DRAM params
    dr = {}
    specs = [
        ("w1t", [97, MO1 * 128], dt.bfloat16),
        ("wih", [128, MOG * MO1 * 128], dt.bfloat16),
        ("whh", [128, MOG * CH * 128], dt.bfloat16),
        ("w2c", [128, MO2 * 3 * 128], dt.bfloat16),
        ("w3s", [128, MO3 * MO2 * 128], dt.bfloat16),
        ("s1", [M + 1, 112], dt.float32),
        ("s2", [96, 2], dt.float32),
        ("bb", [2, 96], dt.float32),
        ("e01", [48, 256], dt.float32),
        ("s4", [128, M], dt.float32),
        ("b2s", [128, MO2], dt.float32),
        ("b3s", [128, MO3], dt.float32),
        ("epsv", [2, 1], dt.float32),
        ("h0b", [128, CH], dt.bfloat16),
        ("h0own", [128, 3], dt.float32),
        ("y", [N, CHUNK], dt.float32),
        ("x01", [M + 1, 1], dt.float32),
        ("xp0", [M, 1], dt.float32),
    ]
    for nm, shp, d in specs:
        dr[nm] = nc.dram_tensor(nm, shp, d, kind="ExternalInput")
    out_d = nc.dram_tensor("out", [M, CHUNK], dt.float32, kind="ExternalOutput")
    hb_o = nc.dram_tensor("hb_o", [128, CH], dt.bfloat16, kind="ExternalOutput")
    ho_o = nc.dram_tensor("ho_o", [128, 3], dt.float32, kind="ExternalOutput")
    xq_o = nc.dram_tensor("xq_o", [M + 1, 1], dt.float32, kind="ExternalOutput")
    xp_o = nc.dram_tensor("xp_o", [M, 1], dt.float32, kind="ExternalOutput")

    with tile.TileContext(nc) as tc:
        with (
            tc.tile_pool(name="w", bufs=1) as wp,
            tc.tile_pool(name="st", bufs=1) as sp,
            tc.tile_pool(name="act", bufs=2) as ap,
            tc.tile_pool(name="ps_big", bufs=1, space="PSUM") as pb,
            tc.tile_pool(name="ps_sm", bufs=1, space="PSUM") as psm,
            tc.tile_pool(name="dram", bufs=1, space="DRAM") as dp,
        ):
            # --- persistent SBUF ---
            w1t = wp.tile([97, MO1 * 128], dt.bfloat16, tag="w1t")
            wih = wp.tile([128, MOG * MO1 * 128], dt.bfloat16, tag="wih")
            whh = wp.tile([128, MOG * CH * 128], dt.bfloat16, tag="whh")
            w2c = wp.tile([128, MO2 * 3 * 128], dt.bfloat16, tag="w2c")
            w3s = wp.tile([128, MO3 * MO2 * 128], dt.bfloat16, tag="w3s")
            s1 = wp.tile([M + 1, 112], dt.float32, tag="s1")
            s2 = wp.tile([96, 2], dt.float32, tag="s2")
            bb = wp.tile([2, 96], dt.float32, tag="bb")
            e01 = wp.tile([48, 256], dt.float32, tag="e01")
            s4 = wp.tile([128, M], dt.float32, tag="s4")
            b2s = wp.tile([128, MO2], dt.float32, tag="b2s")
            b3s = wp.tile([128, MO3], dt.float32, tag="b3s")
            epsv = wp.tile([2, 1], dt.float32, tag="epsv")
            ysb = wp.tile([N, CHUNK], dt.float32, tag="ysb")
            outsb = wp.tile([M, CHUNK], dt.float32, tag="outsb")
            h_blk = sp.tile([128, CH], dt.bfloat16, tag="h_blk")
            h_own = sp.tile([128, 3], dt.float32, tag="h_own")
            xpost1 = sp.tile([M + 1, 1], dt.float32, tag="xpost1")
            xprior = sp.tile([M, 1], dt.float32, tag="xprior")
            send = sp.tile([128, 9], dt.bfloat16, tag="send")
            cc_in = dp.tile([1, 128 * 9], dt.bfloat16, tag="cc_in")
            cc_out = dp.tile([NCORES, 128, 9], dt.bfloat16, tag="cc_out")

            for nm, tl in [("w1t", w1t), ("wih", wih), ("whh", whh), ("w2c", w2c),
                           ("w3s", w3s), ("s1", s1), ("s2", s2), ("bb", bb),
                           ("e01", e01), ("s4", s4), ("b2s", b2s), ("b3s", b3s),
                           ("epsv", epsv), ("y", ysb), ("h0b", h_blk), ("h0own", h_own)]:
                nc.sync.dma_start(tl[:], dr[nm].ap())
            nc.sync.dma_start(xpost1[:], dr["x01"].ap())
            nc.sync.dma_start(xprior[:], dr["xp0"].ap())
            vd = sp.tile([97, 1], dt.float32, tag="vd")
            knet = sp.tile([97, 1], dt.float32, tag="knet")
            knb = sp.tile([97, 1], dt.bfloat16, tag="knb")
            nc.vector.memset(vd[:], 0.0)
            nc.vector.memset(knet[:], 0.0)
            nc.vector.memset(knet[96:97, :], 1.0)
            nc.vector.memset(knb[:], 0.0)
            nc.vector.memset(knb[96:97, :], 1.0)

            def body(t):
                # y column
                y_t = ap.tile([N, 1], dt.float32, tag="y_t")
                nc.sync.dma_start(y_t[:], ysb[:, ds(t, 1)])

                # MM1: pk = [x_prior(4); m1y(48)]
                pk = psm.tile([112, 1], dt.float32, tag="pk")
                nc.tensor.matmul(pk[:], s1[:], xpost1[:], start=True, stop=True)

                # dx then update xprior
                nc.vector.tensor_tensor(vd[64:64 + M, :], xpost1[0:M, :], xprior[:],
                                        op=mybir.AluOpType.subtract)
                nc.scalar.activation(xprior[:], pk[0:M, :], AF.Copy)
                # innov
                nc.vector.tensor_tensor(vd[0:N, :], y_t[:], pk[64:112, :],
                                        op=mybir.AluOpType.subtract)
                sq = ap.tile([96, 1], dt.float32, tag="sq")
                nc.vector.tensor_tensor(sq[:], vd[0:96, :], vd[0:96, :],
                                        op=mybir.AluOpType.mult)
                ss = psm.tile([2, 1], dt.float32, tag="sm3")
                nc.tensor.matmul(ss[:], s2[:], sq[:], start=True, stop=True)
                nrm = ap.tile([2, 1], dt.float32, tag="nrm")
                nc.scalar.activation(nrm[:], ss[:], AF.Sqrt, bias=epsv[:])
                inv = ap.tile([2, 1], dt.float32, tag="inv")
                nc.vector.reciprocal(inv[:], nrm[:])
                ibc = psm.tile([96, 1], dt.float32, tag="sm3")
                nc.tensor.matmul(ibc[:], bb[:], inv[:], start=True, stop=True)
                nc.vector.tensor_tensor(knet[0:96, :], vd[0:96, :], ibc[:],
                                        op=mybir.AluOpType.mult)
                nc.vector.tensor_copy(knb[0:96, :], knet[0:96, :])

                # W1 GEMV -> l1 [128, 33]
                l1p = pb.tile([128, MO1], dt.float32, tag="l1p")
                for m in range(MO1):
                    nc.tensor.matmul(l1p[:, m:m + 1], w1t[:, m * 128:(m + 1) * 128],
                                     knb[:], start=True, stop=True)
                l1b = ap.tile([128, MO1], dt.bfloat16, tag="l1b")
                nc.scalar.activation(l1b[:], l1p[:], AF.Relu)

                # gh = W_hh @ h_blk ; gi = W_ih @ l1
                ghp = pb.tile([128, MOG], dt.float32, tag="ghp")
                for m in range(MOG):
                    for k in range(CH):
                        nc.tensor.matmul(ghp[:, m:m + 1],
                                         whh[:, (m * CH + k) * 128:(m * CH + k + 1) * 128],
                                         h_blk[:, k:k + 1], start=(k == 0), stop=(k == CH - 1))
                gip = pb.tile([128, MOG], dt.float32, tag="gip")
                for m in range(MOG):
                    for k in range(MO1):
                        nc.tensor.matmul(gip[:, m:m + 1],
                                         wih[:, (m * MO1 + k) * 128:(m * MO1 + k + 1) * 128],
                                         l1b[:, k:k + 1], start=(k == 0), stop=(k == MO1 - 1))
                ghs = ap.tile([128, MOG], dt.float32, tag="ghs")
                nc.scalar.activation(ghs[:], ghp[:], AF.Copy)

                # gates
                rzs = ap.tile([128, 6], dt.float32, tag="rzs")
                nc.vector.tensor_tensor(rzs[:], gip[:, 0:6], ghs[:, 0:6],
                                        op=mybir.AluOpType.add)
                rz = ap.tile([128, 6], dt.float32, tag="rz")
                nc.scalar.activation(rz[:], rzs[:], AF.Sigmoid)
                tmp = ap.tile([128, 3], dt.float32, tag="tmp")
                nc.vector.tensor_tensor(tmp[:], rz[:, 0:3], ghs[:, 6:9],
                                        op=mybir.AluOpType.mult)
                nin = ap.tile([128, 3], dt.float32, tag="nin")
                nc.vector.tensor_tensor(nin[:], gip[:, 6:9], tmp[:],
                                        op=mybir.AluOpType.add)
                nt = ap.tile([128, 3], dt.float32, tag="nt")
                nc.scalar.activation(nt[:], nin[:], AF.Tanh)
                dmn = ap.tile([128, 3], dt.float32, tag="dmn")
                nc.vector.tensor_tensor(dmn[:], h_own[:], nt[:], op=mybir.AluOpType.subtract)
                zd = ap.tile([128, 3], dt.float32, tag="zd")
                nc.vector.tensor_tensor(zd[:], rz[:, 3:6], dmn[:], op=mybir.AluOpType.mult)
                nc.vector.tensor_tensor(h_own[:], zd[:], nt[:], op=mybir.AluOpType.add)
                nc.vector.tensor_copy(send[:, 0:3], h_own[:])

                # W2 col-shard partial
                l2pp = pb.tile([128, MO2], dt.float32, tag="bigtmp")
                for m in range(MO2):
                    for k in range(3):
                        nc.tensor.matmul(l2pp[:, m:m + 1],
                                         w2c[:, (m * 3 + k) * 128:(m * 3 + k + 1) * 128],
                                         send[:, k:k + 1], start=(k == 0), stop=(k == 2))
                nc.vector.tensor_copy(send[:, 3:9], l2pp[:])

                # exchange
                nc.sync.dma_start(cc_in[:], send[:])
                nc.gpsimd.collective_compute(
                    "AllGather", mybir.AluOpType.bypass,
                    replica_groups=[list(range(NCORES))],
                    ins=[cc_in.opt()], outs=[cc_out.opt()])
                l2a = ap.tile([128, 48], dt.bfloat16, tag="l2a")
                for cc in range(NCORES):
                    nc.sync.dma_start(h_blk[:, 3 * cc:3 * cc + 3], cc_out[cc, :, 0:3])
                    nc.sync.dma_start(l2a[:, 6 * cc:6 * cc + 6], cc_out[cc, :, 3:9])


                # sum 8 partials -> l2
                t4 = ap.tile([128, 24], dt.float32, tag="t4")
                for i in range(4):
                    nc.vector.tensor_tensor(t4[:, 6 * i:6 * i + 6], l2a[:, 12 * i:12 * i + 6],
                                            l2a[:, 12 * i + 6:12 * i + 12], op=mybir.AluOpType.add)
                t2 = ap.tile([128, 12], dt.float32, tag="t2")
                for i in range(2):
                    nc.vector.tensor_tensor(t2[:, 6 * i:6 * i + 6], t4[:, 12 * i:12 * i + 6],
                                            t4[:, 12 * i + 6:12 * i + 12], op=mybir.AluOpType.add)
                l2s = ap.tile([128, MO2], dt.float32, tag="l2s")
                nc.vector.tensor_tensor(l2s[:], t2[:, 0:6], t2[:, 6:12], op=mybir.AluOpType.add)
                nc.vector.tensor_tensor(l2s[:], l2s[:], b2s[:], op=mybir.AluOpType.add)
                l2b = ap.tile([128, MO2], dt.bfloat16, tag="l2b")
                nc.scalar.activation(l2b[:], l2s[:], AF.Relu)

                # W3 -> kg [128, 2]
                kgp = pb.tile([128, MO3], dt.float32, tag="bigtmp")
                for m in range(MO3):
                    for k in range(MO2):
                        nc.tensor.matmul(kgp[:, m:m + 1],
                                         w3s[:, (m * MO2 + k) * 128:(m * MO2 + k + 1) * 128],
                                         l2b[:, k:k + 1], start=(k == 0), stop=(k == MO2 - 1))
                kgs = ap.tile([128, MO3], dt.float32, tag="kgs")
                nc.vector.tensor_tensor(kgs[:], kgp[:], b3s[:], op=mybir.AluOpType.add)

                # innov broadcast and kg apply
                ib = pb.tile([128, 2], dt.float32, tag="bigtmp")
                nc.tensor.matmul(ib[:, 0:1], e01[:, 0:128], vd[0:N, :], start=True, stop=True)
                nc.tensor.matmul(ib[:, 1:2], e01[:, 128:256], vd[0:N, :], start=True, stop=True)
                prod = ap.tile([128, 2], dt.float32, tag="prod")
                nc.vector.tensor_tensor(prod[:], kgs[:], ib[:], op=mybir.AluOpType.mult)
                xd = psm.tile([M, 2], dt.float32, tag="sm3")
                nc.tensor.matmul(xd[:], s4[:], prod[:], start=True, stop=True)
                xds = ap.tile([M, 2], dt.float32, tag="xds")
                nc.scalar.activation(xds[:], xd[:], AF.Copy)
                txd = ap.tile([M, 1], dt.float32, tag="txd")
                nc.vector.tensor_tensor(txd[:], xds[:, 0:1], xds[:, 1:2], op=mybir.AluOpType.add)
                nc.vector.tensor_tensor(txd[:], txd[:], pk[0:M, :], op=mybir.AluOpType.add)
                nc.vector.tensor_copy(xpost1[0:M, :], txd[:])
                nc.sync.dma_start(outsb[:, ds(t, 1)], txd[:])

            for t in range(CHUNK):
                body(t)

            nc.sync.dma_start(out_d.ap(), outsb[:])
            nc.sync.dma_start(hb_o.ap(), h_blk[:])
            nc.sync.dma_start(ho_o.ap(), h_own[:])
            nc.sync.dma_start(xq_o.ap(), xpost1[:])
            nc.sync.dma_start(xp_o.ap(), xprior[:])
    nc.compile()
    return nc


_CACHE = {}


def kernel(**inputs):
    f32 = np.float32
    inputs = {k: np.asarray(v) for k, v in inputs.items()}
    static = [
        _prep_core(c, inputs["A"], inputs["C"], inputs["x0"], inputs["h0"],
                   inputs["y_seq"], inputs["W1"], inputs["b1"], inputs["W_ih"],
                   inputs["W_hh"], inputs["b_ih"], inputs["b_hh"], inputs["W2"],
                   inputs["b2"], inputs["W3"], inputs["b3"])
        for c in range(NCORES)
    ]
    if "k" not in _CACHE:
        import concourse.bacc as bacc
        nc = bacc.Bacc("TRN2", target_bir_lowering=False, debug=False,
                       num_devices=NCORES)
        _CACHE["k"] = _build(nc)
    nc = _CACHE["k"]
    from concourse import bass_utils

    y = inputs["y_seq"].astype(f32)
    x01 = np.zeros((M + 1, 1), f32)
    x01[:M, 0] = inputs["x0"]
    x01[M, 0] = 1.0
    xp0 = inputs["x0"].reshape(M, 1).astype(f32)
    hb = static[0]["h0b"]
    hown = [st["h0own"] for st in static]

    outs = []
    nch = (NSTEPS + CHUNK - 1) // CHUNK
    for ci in range(nch):
        base = ci * CHUNK
        yc = np.zeros((N, CHUNK), f32)
        seg = y[:, base:base + CHUNK]
        yc[:, :seg.shape[1]] = seg
        in_maps = []
        for c in range(NCORES):
            m = dict(static[c])
            m["y"] = yc
            m["x01"] = x01
            m["xp0"] = xp0
            m["h0b"] = hb
            m["h0own"] = hown[c]
            in_maps.append(m)
        res = bass_utils.run_bass_kernel_spmd(nc, in_maps,
                                              core_ids=list(range(NCORES)))
        r0 = res.results[0]
        outs.append(np.asarray(r0["out"], dtype=f32)[:, :seg.shape[1]])
        hb = np.array(res.results[0]["hb_o"]).astype(BF)
        x01 = np.array(r0["xq_o"], dtype=f32)
        x01[M, 0] = 1.0
        xp0 = np.asarray(r0["xp_o"], dtype=f32)
        hown = []
        for c in range(NCORES):
            ho = np.array(res.results[c]["ho_o"], dtype=f32)
            ho[127, 2] = 1.0
            hown.append(ho)
    return np.concatenate(outs, axis=1)



# revision 11
# speedup vs baseline: 4.1665x; 4.1665x over previous
"""KalmanNetNN Trainium2 kernel: 8-core tensor-parallel, SBUF-resident bf16 weights,
single launch with an on-device For_i loop over all T=512 steps.

Design:
- T=512 strictly sequential steps; per step a chain of GEMVs (W1 4160x52,
  W_ih 6960x4160, W_hh 6960x2320, W2 768x2320, W3 192x768) + tiny Kalman update.
- Weights sharded across 8 cores, resident in SBUF as pre-transposed bf16
  stationary tiles (W-stationary GEMV: out[128,1] tiles land in clean layout).
- GRU hidden (2320) sharded 290/core, padded to 384 slots (3 cols of 128).
- Per step one AllGather exchanges [h_own(384) | l2_partial(768)] bf16;
  W2 is column-sharded so l2 partials sum locally after the AG.
- Small Kalman recurrence (A, C, norms, kg apply) in fp32, replicated on all
  cores (the A-recurrence is unstable; fp32 there keeps rel err ~1e-7).
- The whole sequence runs in ONE device launch: a For_i hardware loop
  (unrolled x2) with y/out accessed via dynamic DRAM slices; weights and all
  recurrent state stay in SBUF for the entire sequence.
"""

import os
import numpy as np
import ml_dtypes

M, N, T = 4, 48, 512
D_IN = M + N            # 52
H1 = 4160               # l1 dim
HID = 2320              # GRU hidden
H2 = 768                # l2 dim
DOUT = M * N            # 192

NCORES = 8
SLOTS = 384             # per-core padded h slots (3 cols of 128)
OWN = HID // NCORES     # 290 real h per core
CH = 3 * NCORES         # 24 global h cols
H1P = 4224              # l1 padded (33 cols); slot (127,32) = bias-1
MO1 = H1P // 128        # 33
MOG = 9                 # gi/gh out cols (3 gates x 3 cols)
MO2 = H2 // 128         # 6
DOP = 256               # padded kg rows
MO3 = DOP // 128        # 2

BF = ml_dtypes.bfloat16
NSTEPS = T


def _tile_stationary(Wc, Mo, C):
    """Wc [Mo*128, C*128] -> [128, Mo*C*128] with tile (m,k) at (m*C+k)*128.
    lhsT[p, j] of tile (m,k) = Wc[128m+j, 128k+p]."""
    A = Wc.reshape(Mo, 128, C, 128)          # m, j, k, p
    A = np.transpose(A, (3, 0, 2, 1))        # p, m, k, j
    return np.ascontiguousarray(A.reshape(128, Mo * C * 128))


def _prep_core(c, A, C_, x0, h0, y_seq, W1, b1, W_ih, W_hh, b_ih, b_hh, W2, b2, W3, b3):
    f32 = np.float32
    out = {}

    # --- W1 | b1: knet layout [97]: dy 0-47, dx 64-67, bias-1 at 96
    W1b = np.zeros((H1P, 97), f32)
    W1b[:H1, 0:N] = W1[:, 0:N]
    W1b[:H1, 64:64 + M] = W1[:, N:D_IN]
    W1b[:H1, 96] = b1
    W1b[H1P - 1, 96] = 1.0   # l1[4223] = relu(1*knet[96]) = 1 -> bias-1 slot
    A1 = W1b.reshape(MO1, 128, 1, 97)
    A1 = np.transpose(A1, (3, 0, 2, 1)).reshape(97, MO1 * 128)
    out["w1t"] = np.ascontiguousarray(A1).astype(BF)

    # --- per-core gate rows: rho = g*384 + s (s<290 real)
    rows = np.zeros((3 * SLOTS,), np.int64) - 1
    for g in range(3):
        for s in range(OWN):
            rows[g * SLOTS + s] = g * HID + c * OWN + s
    valid = rows >= 0

    # --- W_ih core [1152, H1P] + b_ih in col 4223 (l1 bias-1 slot)
    Wih = np.zeros((3 * SLOTS, H1P), f32)
    Wih[valid, :H1] = W_ih[rows[valid]]
    Wih[valid, H1P - 1] = b_ih[rows[valid]]
    Wih[SLOTS + 383, H1P - 1] = 30.0  # z-gate bias at dead slot s=383 -> z=1, h slot stays 1
    out["wih"] = _tile_stationary(Wih, MOG, MO1).astype(BF)

    # --- global h slot map: slot (cc, j, p) -> h index cc*290 + 128j + p (if <290)
    hidx = np.zeros((CH, 128), np.int64) - 1    # [col, p]
    for cc in range(NCORES):
        for j in range(3):
            for p in range(128):
                s = 128 * j + p
                if s < OWN:
                    hidx[3 * cc + j, p] = cc * OWN + s
    # --- W_hh core [1152, CH*128] + b_hh at slot col 23, p 127 (h bias-1)
    Whh = np.zeros((3 * SLOTS, CH * 128), f32)
    flat = hidx.reshape(-1)                      # [CH*128] in (col, p) order
    vv = flat >= 0
    Whh[np.ix_(valid, vv)] = W_hh[np.ix_(rows[valid], flat[vv])]
    Whh[valid, 23 * 128 + 127] = b_hh[rows[valid]]
    out["whh"] = _tile_stationary(Whh, MOG, CH).astype(BF)

    # --- W2 column shard: own slots only [768, 3*128]
    W2c = np.zeros((H2, 3 * 128), f32)
    for j in range(3):
        for p in range(128):
            s = 128 * j + p
            if s < OWN:
                W2c[:, j * 128 + p] = W2[:, c * OWN + s]
    out["w2c"] = _tile_stationary(W2c, MO2, 3).astype(BF)

    # --- W3: rows rho=4n+m <-> W3 row m*N+n, x 1e-4 fold
    W3s = np.zeros((DOP, H2), f32)
    for rho in range(DOUT):
        n_, m_ = rho // 4, rho % 4
        W3s[rho] = W3[m_ * N + n_] * 1e-4
    out["w3s"] = _tile_stationary(W3s, MO3, MO2).astype(BF)

    # --- small fp32 constants
    CA = (C_[:, :M] @ A).astype(f32)
    c5 = C_[:, M].astype(f32)
    S1 = np.zeros((M + 1, 112), f32)   # pk: x_prior @ 0-3, m1y @ 64-111
    S1[:M, :M] = A.T
    S1[:M, 64:] = CA.T
    S1[M, 64:] = c5
    out["s1"] = S1
    S2 = np.zeros((96, 2), f32)
    S2[:N, 0] = 1.0
    S2[64:64 + M, 1] = 1.0
    out["s2"] = S2
    BB = np.zeros((2, 96), f32)
    BB[0, :N] = 1.0
    BB[1, 64:64 + M] = 1.0
    out["bb"] = BB
    E = np.zeros((DOP, 48), f32)
    for rho in range(DOUT):
        E[rho, rho // 4] = 1.0
    out["e01"] = np.ascontiguousarray(E.reshape(2, 128, 48).transpose(2, 0, 1).reshape(48, 256))
    S4 = np.zeros((128, M), f32)
    for p in range(128):
        S4[p, p % 4] = 1.0
    out["s4"] = S4
    b2s = np.zeros((128, MO2), f32)
    b2s[:, :] = b2.reshape(MO2, 128).T
    out["b2s"] = b2s
    b3v = np.zeros((DOP,), f32)
    for rho in range(DOUT):
        n_, m_ = rho // 4, rho % 4
        b3v[rho] = b3[m_ * N + n_] * 1e-4
    out["b3s"] = np.ascontiguousarray(b3v.reshape(MO3, 128).T)
    out["epsv"] = np.full((2, 1), 1e-24, f32)

    # --- h0 blocks (all cores' slots) bf16 + own fp32
    h0b = np.zeros((128, CH), f32)
    for cc in range(CH):
        for p in range(128):
            if hidx[cc, p] >= 0:
                h0b[p, cc] = h0[hidx[cc, p]]
    h0b[127, 23] = 1.0  # bias-1 slot
    out["h0b"] = h0b.astype(BF)
    own = np.ascontiguousarray(h0b[:, 3 * c:3 * c + 3]).astype(f32)
    own[127, 2] = 1.0
    out["h0own"] = own
    return out


def _build(nc, nsteps):
    import concourse.bass as bass
    import concourse.mybir as mybir
    import concourse.tile as tile

    dt = mybir.dt
    AF = mybir.ActivationFunctionType
    ds = bass.ds

    # DRAM params
    dr = {}
    specs = [
        ("w1t", [97, MO1 * 128], dt.bfloat16),
        ("wih", [128, MOG * MO1 * 128], dt.bfloat16),
        ("whh", [128, MOG * CH * 128], dt.bfloat16),
        ("w2c", [128, MO2 * 3 * 128], dt.bfloat16),
        ("w3s", [128, MO3 * MO2 * 128], dt.bfloat16),
        ("s1", [M + 1, 112], dt.float32),
        ("s2", [96, 2], dt.float32),
        ("bb", [2, 96], dt.float32),
        ("e01", [48, 256], dt.float32),
        ("s4", [128, M], dt.float32),
        ("b2s", [128, MO2], dt.float32),
        ("b3s", [128, MO3], dt.float32),
        ("epsv", [2, 1], dt.float32),
        ("h0b", [128, CH], dt.bfloat16),
        ("h0own", [128, 3], dt.float32),
        ("y", [N, T], dt.float32),
        ("x01", [M + 1, 1], dt.float32),
        ("xp0", [M, 1], dt.float32),
    ]
    for nm, shp, d in specs:
        dr[nm] = nc.dram_tensor(nm, shp, d, kind="ExternalInput")
    out_d = nc.dram_tensor("out", [M, T], dt.float32, kind="ExternalOutput")

    with tile.TileContext(nc) as tc:
        with (
            tc.tile_pool(name="w", bufs=1) as wp,
            tc.tile_pool(name="st", bufs=1) as sp,
            tc.tile_pool(name="act", bufs=2) as ap,
            tc.tile_pool(name="ps_big", bufs=1, space="PSUM") as pb,
            tc.tile_pool(name="ps_sm", bufs=1, space="PSUM") as psm,
            tc.tile_pool(name="dram", bufs=1, space="DRAM") as dp,
        ):
            # --- persistent SBUF ---
            w1t = wp.tile([97, MO1 * 128], dt.bfloat16, tag="w1t")
            wih = wp.tile([128, MOG * MO1 * 128], dt.bfloat16, tag="wih")
            whh = wp.tile([128, MOG * CH * 128], dt.bfloat16, tag="whh")
            w2c = wp.tile([128, MO2 * 3 * 128], dt.bfloat16, tag="w2c")
            w3s = wp.tile([128, MO3 * MO2 * 128], dt.bfloat16, tag="w3s")
            s1 = wp.tile([M + 1, 112], dt.float32, tag="s1")
            s2 = wp.tile([96, 2], dt.float32, tag="s2")
            bb = wp.tile([2, 96], dt.float32, tag="bb")
            e01 = wp.tile([48, 256], dt.float32, tag="e01")
            s4 = wp.tile([128, M], dt.float32, tag="s4")
            b2s = wp.tile([128, MO2], dt.float32, tag="b2s")
            b3s = wp.tile([128, MO3], dt.float32, tag="b3s")
            epsv = wp.tile([2, 1], dt.float32, tag="epsv")
            h_blk = sp.tile([128, CH], dt.bfloat16, tag="h_blk")
            h_own = sp.tile([128, 3], dt.float32, tag="h_own")
            xpost1 = sp.tile([M + 1, 1], dt.float32, tag="xpost1")
            xprior = sp.tile([M, 1], dt.float32, tag="xprior")
            send = sp.tile([128, 9], dt.bfloat16, tag="send")
            cc_in = dp.tile([1, 128 * 9], dt.bfloat16, tag="cc_in")
            cc_out = dp.tile([NCORES, 128, 9], dt.bfloat16, tag="cc_out")

            for nm, tl in [("w1t", w1t), ("wih", wih), ("whh", whh), ("w2c", w2c),
                           ("w3s", w3s), ("s1", s1), ("s2", s2), ("bb", bb),
                           ("e01", e01), ("s4", s4), ("b2s", b2s), ("b3s", b3s),
                           ("epsv", epsv), ("h0b", h_blk), ("h0own", h_own)]:
                nc.sync.dma_start(tl[:], dr[nm].ap())
            nc.sync.dma_start(xpost1[:], dr["x01"].ap())
            nc.sync.dma_start(xprior[:], dr["xp0"].ap())
            vd = sp.tile([97, 1], dt.float32, tag="vd")
            knet = sp.tile([97, 1], dt.float32, tag="knet")
            knb = sp.tile([97, 1], dt.bfloat16, tag="knb")
            nc.vector.memset(vd[:], 0.0)
            nc.vector.memset(knet[:], 0.0)
            nc.vector.memset(knet[96:97, :], 1.0)
            nc.vector.memset(knb[:], 0.0)
            nc.vector.memset(knb[96:97, :], 1.0)

            def body(t_ex, y_t):
                # MM1: pk = [x_prior(4); m1y(48)]
                pk = psm.tile([112, 1], dt.float32, tag="pk")
                nc.tensor.matmul(pk[:], s1[:], xpost1[:], start=True, stop=True)

                # gh = W_hh @ h_blk -- emitted early/interleaved so PE chews on
                # it while DVE/ACT run the small knet chain.
                ghp = pb.tile([128, MOG], dt.float32, tag="ghp")

                def gh_chunk(m):
                    for k in range(CH):
                        nc.tensor.matmul(ghp[:, m:m + 1],
                                         whh[:, (m * CH + k) * 128:(m * CH + k + 1) * 128],
                                         h_blk[:, k:k + 1], start=(k == 0), stop=(k == CH - 1))

                gh_chunk(0)

                # dx then update xprior
                nc.vector.tensor_tensor(vd[64:64 + M, :], xpost1[0:M, :], xprior[:],
                                        op=mybir.AluOpType.subtract)
                nc.scalar.activation(xprior[:], pk[0:M, :], AF.Copy)
                # innov
                nc.vector.tensor_tensor(vd[0:N, :], y_t, pk[64:112, :],
                                        op=mybir.AluOpType.subtract)
                sq = ap.tile([96, 1], dt.float32, tag="sq")
                nc.vector.tensor_tensor(sq[:], vd[0:96, :], vd[0:96, :],
                                        op=mybir.AluOpType.mult)
                ss = psm.tile([2, 1], dt.float32, tag="sm3")
                nc.tensor.matmul(ss[:], s2[:], sq[:], start=True, stop=True)
                gh_chunk(1)
                nrm = ap.tile([2, 1], dt.float32, tag="nrm")
                nc.scalar.activation(nrm[:], ss[:], AF.Sqrt, bias=epsv[:])
                inv = ap.tile([2, 1], dt.float32, tag="inv")
                nc.vector.reciprocal(inv[:], nrm[:])
                ibc = psm.tile([96, 1], dt.float32, tag="sm3")
                nc.tensor.matmul(ibc[:], bb[:], inv[:], start=True, stop=True)
                # innov broadcast for the kg apply at step end (innov ready now)
                ib = psm.tile([128, 2], dt.float32, tag="ib")
                nc.tensor.matmul(ib[:, 0:1], e01[:, 0:128], vd[0:N, :], start=True, stop=True)
                nc.tensor.matmul(ib[:, 1:2], e01[:, 128:256], vd[0:N, :], start=True, stop=True)
                gh_chunk(2)
                nc.vector.tensor_tensor(knet[0:96, :], vd[0:96, :], ibc[:],
                                        op=mybir.AluOpType.mult)
                nc.vector.tensor_copy(knb[0:96, :], knet[0:96, :])
                for m in range(3, MOG):
                    gh_chunk(m)

                # W1 GEMV -> l1 [128, 33]
                l1p = pb.tile([128, MO1], dt.float32, tag="l1p")
                for m in range(MO1):
                    nc.tensor.matmul(l1p[:, m:m + 1], w1t[:, m * 128:(m + 1) * 128],
                                     knb[:], start=True, stop=True)
                l1b = ap.tile([128, MO1], dt.bfloat16, tag="l1b")
                nc.scalar.activation(l1b[:], l1p[:], AF.Relu)

                # gi = W_ih @ l1
                gip = pb.tile([128, MOG], dt.float32, tag="gip")
                for m in range(MOG):
                    for k in range(MO1):
                        nc.tensor.matmul(gip[:, m:m + 1],
                                         wih[:, (m * MO1 + k) * 128:(m * MO1 + k + 1) * 128],
                                         l1b[:, k:k + 1], start=(k == 0), stop=(k == MO1 - 1))
                ghs = ap.tile([128, MOG], dt.float32, tag="ghs")
                nc.scalar.activation(ghs[:], ghp[:], AF.Copy)

                # gates
                rzs = ap.tile([128, 6], dt.float32, tag="rzs")
                nc.vector.tensor_tensor(rzs[:], gip[:, 0:6], ghs[:, 0:6],
                                        op=mybir.AluOpType.add)
                rz = ap.tile([128, 6], dt.float32, tag="rz")
                nc.scalar.activation(rz[:], rzs[:], AF.Sigmoid)
                tmp = ap.tile([128, 3], dt.float32, tag="tmp")
                nc.vector.tensor_tensor(tmp[:], rz[:, 0:3], ghs[:, 6:9],
                                        op=mybir.AluOpType.mult)
                nin = ap.tile([128, 3], dt.float32, tag="nin")
                nc.vector.tensor_tensor(nin[:], gip[:, 6:9], tmp[:],
                                        op=mybir.AluOpType.add)
                nt = ap.tile([128, 3], dt.float32, tag="nt")
                nc.scalar.activation(nt[:], nin[:], AF.Tanh)
                dmn = ap.tile([128, 3], dt.float32, tag="dmn")
                nc.vector.tensor_tensor(dmn[:], h_own[:], nt[:], op=mybir.AluOpType.subtract)
                zd = ap.tile([128, 3], dt.float32, tag="zd")
                nc.vector.tensor_tensor(zd[:], rz[:, 3:6], dmn[:], op=mybir.AluOpType.mult)
                nc.vector.tensor_tensor(h_own[:], zd[:], nt[:], op=mybir.AluOpType.add)
                nc.vector.tensor_copy(send[:, 0:3], h_own[:])

                # W2 col-shard partial
                l2pp = pb.tile([128, MO2], dt.float32, tag="bigtmp")
                for m in range(MO2):
                    for k in range(3):
                        nc.tensor.matmul(l2pp[:, m:m + 1],
                                         w2c[:, (m * 3 + k) * 128:(m * 3 + k + 1) * 128],
                                         send[:, k:k + 1], start=(k == 0), stop=(k == 2))
                nc.vector.tensor_copy(send[:, 3:9], l2pp[:])

                # exchange
                nc.sync.dma_start(cc_in[:], send[:])
                if os.environ.get("KNOCC", "0") == "1":
                    # bisect mode: skip the collective, fan the local send into
                    # every rank slot of cc_out (wrong numbers, same dataflow)
                    for cc in range(NCORES):
                        nc.sync.dma_start(cc_out[cc, :, :], send[:])
                else:
                    nc.gpsimd.collective_compute(
                        "AllGather", mybir.AluOpType.bypass,
                        replica_groups=[list(range(NCORES))],
                        ins=[cc_in.opt()], outs=[cc_out.opt()])
                l2a = ap.tile([128, 48], dt.bfloat16, tag="l2a")
                for cc in range(NCORES):
                    nc.gpsimd.dma_start(h_blk[:, 3 * cc:3 * cc + 3], cc_out[cc, :, 0:3])
                    nc.gpsimd.dma_start(l2a[:, 6 * cc:6 * cc + 6], cc_out[cc, :, 3:9])

                # sum 8 partials -> l2
                t4 = ap.tile([128, 24], dt.float32, tag="t4")
                for i in range(4):
                    nc.vector.tensor_tensor(t4[:, 6 * i:6 * i + 6], l2a[:, 12 * i:12 * i + 6],
                                            l2a[:, 12 * i + 6:12 * i + 12], op=mybir.AluOpType.add)
                t2 = ap.tile([128, 12], dt.float32, tag="t2")
                for i in range(2):
                    nc.vector.tensor_tensor(t2[:, 6 * i:6 * i + 6], t4[:, 12 * i:12 * i + 6],
                                            t4[:, 12 * i + 6:12 * i + 12], op=mybir.AluOpType.add)
                l2s = ap.tile([128, MO2], dt.float32, tag="l2s")
                nc.vector.tensor_tensor(l2s[:], t2[:, 0:6], t2[:, 6:12], op=mybir.AluOpType.add)
                nc.vector.tensor_tensor(l2s[:], l2s[:], b2s[:], op=mybir.AluOpType.add)
                l2b = ap.tile([128, MO2], dt.bfloat16, tag="l2b")
                nc.scalar.activation(l2b[:], l2s[:], AF.Relu)

                # W3 -> kg [128, 2]
                kgp = pb.tile([128, MO3], dt.float32, tag="bigtmp")
                for m in range(MO3):
                    for k in range(MO2):
                        nc.tensor.matmul(kgp[:, m:m + 1],
                                         w3s[:, (m * MO2 + k) * 128:(m * MO2 + k + 1) * 128],
                                         l2b[:, k:k + 1], start=(k == 0), stop=(k == MO2 - 1))
                kgs = ap.tile([128, MO3], dt.float32, tag="kgs")
                nc.vector.tensor_tensor(kgs[:], kgp[:], b3s[:], op=mybir.AluOpType.add)

                # kg apply
                prod = ap.tile([128, 2], dt.float32, tag="prod")
                nc.vector.tensor_tensor(prod[:], kgs[:], ib[:], op=mybir.AluOpType.mult)
                xd = psm.tile([M, 2], dt.float32, tag="sm3")
                nc.tensor.matmul(xd[:], s4[:], prod[:], start=True, stop=True)
                xds = ap.tile([M, 2], dt.float32, tag="xds")
                nc.scalar.activation(xds[:], xd[:], AF.Copy)
                txd = ap.tile([M, 1], dt.float32, tag="txd")
                nc.vector.tensor_tensor(txd[:], xds[:, 0:1], xds[:, 1:2], op=mybir.AluOpType.add)
                nc.vector.tensor_tensor(xpost1[0:M, :], txd[:], pk[0:M, :],
                                        op=mybir.AluOpType.add)
                nc.gpsimd.dma_start(out_d.ap()[:, ds(t_ex, 1)], xpost1[0:M, :])

            assert nsteps % 2 == 0
            hint = ()
            if os.environ.get("KHINT", "0") == "1":
                hint = (mybir.EngineType.PE,)
            with tc.For_i(0, nsteps, 2, hint_engines=hint) as t0:
                y2 = ap.tile([N, 2], dt.float32, tag="y2")
                nc.gpsimd.dma_start(y2[:], dr["y"].ap()[:, ds(t0, 2)])
                body(t0, y2[:, 0:1])
                body(t0 + 1, y2[:, 1:2])
    nc.compile()
    return nc


_CACHE = {}


def kernel(**inputs):
    f32 = np.float32
    inputs = {k: np.asarray(v) for k, v in inputs.items()}
    static = [
        _prep_core(c, inputs["A"], inputs["C"], inputs["x0"], inputs["h0"],
                   inputs["y_seq"], inputs["W1"], inputs["b1"], inputs["W_ih"],
                   inputs["W_hh"], inputs["b_ih"], inputs["b_hh"], inputs["W2"],
                   inputs["b2"], inputs["W3"], inputs["b3"])
        for c in range(NCORES)
    ]
    nsteps = NSTEPS
    key = ("k", nsteps)
    if key not in _CACHE:
        import concourse.bacc as bacc
        nc = bacc.Bacc("TRN2", target_bir_lowering=False, debug=False,
                       num_devices=NCORES)
        _CACHE[key] = _build(nc, nsteps)
    nc = _CACHE[key]
    from concourse import bass_utils

    y = np.zeros((N, T), f32)
    ys = inputs["y_seq"].astype(f32)
    y[:, :ys.shape[1]] = ys[:, :T]
    x01 = np.zeros((M + 1, 1), f32)
    x01[:M, 0] = inputs["x0"]
    x01[M, 0] = 1.0
    xp0 = inputs["x0"].reshape(M, 1).astype(f32)

    in_maps = []
    for c in range(NCORES):
        m = dict(static[c])
        m["y"] = y
        m["x01"] = x01
        m["xp0"] = xp0
        in_maps.append(m)
    trace = os.environ.get("BASS_TRACE_RUN") == "1"
    res = bass_utils.run_bass_kernel_spmd(nc, in_maps,
                                          core_ids=list(range(NCORES)),
                                          trace=trace)
    _CACHE["last_result"] = res
    out = np.asarray(res.results[0]["out"], dtype=f32)
    return out[:, :nsteps]
